# revision 4
# baseline (speedup 1.0000x reference)
"""Trainium2 Bass kernel for NeuromodulatedHolographicBrain.

Math (reference):
    r_gate  = sigmoid(x @ router_w.T + router_b)            # [B, 64]
    mask    = repeat(r_gate, 64, axis=1)                    # [B, H]
    sensory = (x @ W + bW) * mask                           # W from COO edges
    rec     = h_prev @ R + bR
    target  = tanh(sensory + rec)
    h_new   = h_prev + gate * (target - h_prev) * (DT/tau)
    pred    = h_new @ P + bP
    return (h_new, pred)

Strategy: densify the 1%-sparse edge-list weights on the host, then run
dense fp32r matmuls on the PE array. Hidden dim (4096) is column-sharded
across 8 cores (512 cols each): each core reads x^T and h_prev^T in full,
its own W/R column slabs and P row slab, computes its h_new^T shard and a
full [H, B] pred^T partial (contraction over its h_new shard); the host
sums the 8 partials. Everything on-chip is in transposed layout
[features(partitions), batch(free)] so no device transposes are needed.
"""

import numpy as np

B = 512
IN = 2048
H = 4096
SH = 512          # hidden cols per core
NCORES = 8
KA = IN // 128    # 16  K-tiles for x contraction
KC = H // 128     # 32  K-tiles for h contraction
M = SH // 128     # 4   m-tiles per shard
F = H // 128      # 32  f-tiles for pred output
DT = 0.1
RB = 64           # router blocks
RSH = RB // NCORES  # 8 router blocks per core

_prog = None


def _legalize_waits(nc, mybir, max_waits=1):
    """Split multi-wait instructions into single-wait NoOps.

    The walrus build here rejects >1 piggybacked sync wait on (at least)
    S3_LW-lowered matmuls and Drains.
    """
    ctr = 0
    n_split = 0
    for f in nc.m.functions:
        for blk in f.blocks:
            out = []
            for ins in blk.instructions:
                si = ins.sync_info
                if si is not None and len(si.on_wait) > max_waits:
                    waits = list(si.on_wait)
                    extra, keep = waits[:-max_waits], waits[-max_waits:]
                    for w in extra:
                        ctr += 1
                        nop = mybir.InstNoOp(name=f"waitnop-{ctr}")
                        nop.engine = ins.engine
                        nop.sync_info = mybir.SyncInfo(on_wait=[w], on_update=[])
                        out.append(nop)
                        n_split += 1
                    si.on_wait = keep
                out.append(ins)
            blk.instructions[:] = out
    return n_split


def _build_program():
    import concourse.bass as bass
    import concourse.mybir as mybir
    import concourse.tile as tile

    f32 = mybir.dt.float32
    f32r = mybir.dt.float32r
    Alu = mybir.AluOpType
    Act = mybir.ActivationFunctionType

    nc = bass.Bass()

    awr_d = nc.dram_tensor("awr", [KA, 128, 1032], f32r, kind="ExternalInput")
    hr_d = nc.dram_tensor("hr", [KC, 128, 1024], f32r, kind="ExternalInput")
    p_d = nc.dram_tensor("p", [M, 128, H], f32r, kind="ExternalInput")
    hps_d = nc.dram_tensor("hps", [M, 128, B], f32, kind="ExternalInput")
    g_d = nc.dram_tensor("g", [M, 128, B], f32, kind="ExternalInput")
    eb_d = nc.dram_tensor("eb", [RSH, B], f32r, kind="ExternalInput")
    bias_d = nc.dram_tensor("bias", [128, 2 * M + 1], f32, kind="ExternalInput")
    hn_d = nc.dram_tensor("hn", [M, 128, B], f32, kind="ExternalOutput")
    pp_d = nc.dram_tensor("pp", [F, 128, B], f32, kind="ExternalOutput")

    with tile.TileContext(nc) as tc:
        with (
            tc.tile_pool(name="consts", bufs=1) as consts,
            tc.tile_pool(name="astream", bufs=3) as astream,
            tc.tile_pool(name="cstream", bufs=3) as cstream,
            tc.tile_pool(name="pres", bufs=1) as pres,
            tc.tile_pool(name="sens", bufs=1) as senspool,
            tc.tile_pool(name="hn", bufs=1) as hnpool,
            tc.tile_pool(name="tmp", bufs=3) as tmppool,
            tc.tile_pool(name="acc", bufs=4, space="PSUM") as acc_pool,
            tc.tile_pool(name="psb", bufs=2, space="PSUM") as psb_pool,
            tc.tile_pool(name="psr", bufs=1, space="PSUM") as psr_pool,
        ):
            # ---- constants ----
            eb_t = consts.tile([RSH, B], f32r, tag="eb")
            nc.sync.dma_start(eb_t[:], eb_d[:])
            bias_t = consts.tile([128, 2 * M + 1], f32, tag="bias")
            nc.sync.dma_start(bias_t[:], bias_d[:])
            hps_t = consts.tile([128, M, B], f32, tag="hps")
            g_t = consts.tile([128, M, B], f32, tag="g")
            for m in range(M):
                nc.sync.dma_start(hps_t[:, m, :], hps_d[m])
                nc.sync.dma_start(g_t[:, m, :], g_d[m])

            # ---- phase A: router + sensory accumulation over x K-tiles ----
            rg_ps = psr_pool.tile([RSH, B], f32, tag="rg")
            s_ps = [acc_pool.tile([128, B], f32, tag="acc", name=f"s_ps{i}") for i in range(M)]
            for k in range(KA):
                a_t = astream.tile([128, 1032], f32r, tag="awr")
                nc.sync.dma_start(a_t[:], awr_d[k])
                xt = a_t[:, 0:B]
                nc.tensor.matmul(rg_ps[:], a_t[:, 1024:1032], xt,
                                 start=(k == 0), stop=(k == KA - 1))
                for m in range(M):
                    nc.tensor.matmul(s_ps[m][:], a_t[:, B + 128 * m:B + 128 * (m + 1)],
                                     xt, start=(k == 0), stop=(k == KA - 1))

            # ---- phase B: sigmoid -> mask expand -> masked sensory drain ----
            rg32 = tmppool.tile([RSH, B], f32, tag="rg32")
            nc.scalar.activation(rg32[:], rg_ps[:], Act.Sigmoid,
                                 bias=bias_t[0:RSH, 2 * M:2 * M + 1], scale=1.0)
            rg_r = tmppool.tile([RSH, B], f32r, tag="rgr")
            nc.vector.tensor_copy(rg_r[:], rg32[:])

            sens = []
            for m in range(M):
                mask_ps = psb_pool.tile([128, B], f32, tag="mask")
                nc.tensor.matmul(mask_ps[:], eb_t[:, 128 * m:128 * (m + 1)], rg_r[:],
                                 start=True, stop=True)
                mask_sb = tmppool.tile([128, B], f32, tag="masksb")
                nc.scalar.activation(mask_sb[:], mask_ps[:], Act.Copy)
                s_sb = senspool.tile([128, B], f32, tag=f"sens{m}")
                # (x@W + bW) * mask
                nc.vector.scalar_tensor_tensor(
                    s_sb[:], s_ps[m][:], bias_t[:, m:m + 1], mask_sb[:],
                    op0=Alu.add, op1=Alu.mult)
                sens.append(s_sb)

            # ---- phase C: rec accumulation over h_prev K-tiles ----
            rec_ps = [acc_pool.tile([128, B], f32, tag="acc", name=f"rec_ps{i}") for i in range(M)]
            p_t = pres.tile([128, M, H], f32r, tag="p")
            for k in range(KC):
                c_t = cstream.tile([128, 1024], f32r, tag="hr")
                nc.sync.dma_start(c_t[:], hr_d[k])
                ht = c_t[:, 0:B]
                for m in range(M):
                    nc.tensor.matmul(rec_ps[m][:], c_t[:, B + 128 * m:B + 128 * (m + 1)],
                                     ht, start=(k == 0), stop=(k == KC - 1))
                # spread the 8MB P-slab load across phase C (1MB chunks)
                if k % 4 == 0:
                    chunk = k // 4  # 0..7
                    kb, h = divmod(chunk, 2)
                    nc.sync.dma_start(
                        p_t[:, kb, 2048 * h:2048 * (h + 1)],
                        p_d[kb][:, 2048 * h:2048 * (h + 1)])

            # ---- phase D: target, h_new ----
            hn_ts = []
            for m in range(M):
                tmp = tmppool.tile([128, B], f32, tag="dtmp")
                # (rec + bR) + sens
                nc.vector.scalar_tensor_tensor(
                    tmp[:], rec_ps[m][:], bias_t[:, M + m:M + m + 1], sens[m][:],
                    op0=Alu.add, op1=Alu.add)
                tgt = tmppool.tile([128, B], f32, tag="dtgt")
                nc.scalar.activation(tgt[:], tmp[:], Act.Tanh)
                d_sb = tmppool.tile([128, B], f32, tag="dd")
                nc.vector.tensor_sub(d_sb[:], tgt[:], hps_t[:, m, :])
                e_sb = tmppool.tile([128, B], f32, tag="de")
                nc.vector.tensor_mul(e_sb[:], d_sb[:], g_t[:, m, :])
                hn_sb = hnpool.tile([128, B], f32, tag=f"hn{m}")
                nc.vector.tensor_add(hn_sb[:], e_sb[:], hps_t[:, m, :])
                nc.sync.dma_start(hn_d[m], hn_sb[:])
                hn_r = hnpool.tile([128, B], f32r, tag=f"hnr{m}")
                nc.vector.tensor_copy(hn_r[:], hn_sb[:])
                hn_ts.append(hn_r)

            # ---- phase E: pred partial = P[shard,:]^T @ h_new^T_shard ----
            for f in range(F):
                pp_ps = acc_pool.tile([128, B], f32, tag="acc", name=f"pp_ps{f}")
                for kb in range(M):
                    nc.tensor.matmul(pp_ps[:], p_t[:, kb, 128 * f:128 * (f + 1)],
                                     hn_ts[kb][:], start=(kb == 0), stop=(kb == M - 1))
                pp_sb = tmppool.tile([128, B], f32, tag="ppsb", name=f"pp_sb{f}")
                nc.scalar.activation(pp_sb[:], pp_ps[:], Act.Copy)
                nc.sync.dma_start(pp_d[f], pp_sb[:])

    _legalize_waits(nc, mybir)
    nc.finalize()
    return nc


def _get_program():
    global _prog
    if _prog is None:
        _prog = _build_program()
    return _prog


def _densify(rows, cols, vals, n_rows, n_cols):
    flat = rows.astype(np.int64) * n_cols + cols.astype(np.int64)
    dense = np.bincount(flat, weights=vals.astype(np.float64),
                        minlength=n_rows * n_cols)
    return dense.astype(np.float32).reshape(n_rows, n_cols)


def kernel(x, h_prev, gate, W_vals, W_bias, R_vals, R_bias, P_vals, P_bias,
           router_w, router_b, tau, W_rows, W_cols, R_rows, R_cols,
           P_rows, P_cols):
    from concourse.bass_utils import run_bass_kernel_spmd

    x = np.asarray(x, np.float32)
    h_prev = np.asarray(h_prev, np.float32)
    gate = np.asarray(gate, np.float32)

    Wd = _densify(np.asarray(W_rows), np.asarray(W_cols), np.asarray(W_vals), IN, H)
    Rd = _densify(np.asarray(R_rows), np.asarray(R_cols), np.asarray(R_vals), H, H)
    Pd = _densify(np.asarray(P_rows), np.asarray(P_cols), np.asarray(P_vals), H, H)

    XT = np.ascontiguousarray(x.T).reshape(KA, 128, B)              # [16,128,512]
    HpT = np.ascontiguousarray(h_prev.T).reshape(KC, 128, B)        # [32,128,512]
    rwT = np.ascontiguousarray(np.asarray(router_w, np.float32).T)  # [2048, 64]
    dt_tau = (DT / np.asarray(tau, np.float32))                     # [4096]
    gate1 = gate.reshape(B)

    ebmat = np.zeros((RSH, B), np.float32)
    for j in range(RSH):
        ebmat[j, 64 * j:64 * (j + 1)] = 1.0

    in_maps = []
    for c in range(NCORES):
        sh = slice(SH * c, SH * (c + 1))
        w_slab = np.ascontiguousarray(Wd[:, sh]).reshape(KA, 128, SH)
        r_slab = np.ascontiguousarray(Rd[:, sh]).reshape(KC, 128, SH)
        p_slab = np.ascontiguousarray(Pd[sh, :]).reshape(M, 128, H)
        rw_slab = np.ascontiguousarray(
            rwT[:, RSH * c:RSH * (c + 1)]).reshape(KA, 128, RSH)
        awr = np.concatenate([XT, w_slab, rw_slab], axis=2)         # [16,128,1032]
        hr = np.concatenate([HpT, r_slab], axis=2)                  # [32,128,1024]
        hps = np.ascontiguousarray(h_prev.T[sh]).reshape(M, 128, B)
        g = np.ascontiguousarray(
            np.outer(dt_tau[sh], gate1)).reshape(M, 128, B)
        bias = np.zeros((128, 2 * M + 1), np.float32)
        bias[:, 0:M] = np.asarray(W_bias, np.float32)[sh].reshape(M, 128).T
        bias[:, M:2 * M] = np.asarray(R_bias, np.float32)[sh].reshape(M, 128).T
        bias[0:RSH, 2 * M] = np.asarray(router_b, np.float32)[RSH * c:RSH * (c + 1)]
        in_maps.append({
            "awr": np.ascontiguousarray(awr),
            "hr": np.ascontiguousarray(hr),
            "p": p_slab,
            "hps": hps,
            "g": g,
            "eb": ebmat,
            "bias": bias,
        })

    nc = _get_program()
    res = run_bass_kernel_spmd(nc, in_maps, list(range(NCORES)))

    h_new = np.empty((B, H), np.float32)
    predT = np.zeros((H, B), np.float64)
    for c in range(NCORES):
        sh = slice(SH * c, SH * (c + 1))
        h_new[:, sh] = res.results[c]["hn"].reshape(SH, B).T
        predT += res.results[c]["pp"].reshape(H, B)
    pred = predT.T.astype(np.float32) + np.asarray(P_bias, np.float32)
    return (h_new, pred)


# revision 7
# speedup vs baseline: 1.2638x; 1.2638x over previous
"""Trainium2 Bass kernel for NeuromodulatedHolographicBrain.

Math (reference):
    r_gate  = sigmoid(x @ router_w.T + router_b)            # [B, 64]
    mask    = repeat(r_gate, 64, axis=1)                    # [B, H]
    sensory = (x @ W + bW) * mask                           # W from COO edges
    rec     = h_prev @ R + bR
    target  = tanh(sensory + rec)
    h_new   = h_prev + gate * (target - h_prev) * (DT/tau)
    pred    = h_new @ P + bP
    return (h_new, pred)

Strategy: densify the 1%-sparse edge-list weights on the host, then run
dense fp32r matmuls on the PE array. Hidden dim (4096) is column-sharded
across 8 cores (512 cols each): each core reads x^T and h_prev^T in full,
its own W/R column slabs and P row slab, computes its h_new^T shard and a
full [H, B] pred^T partial (contraction over its h_new shard); the host
sums the 8 partials. Everything on-chip is in transposed layout
[features(partitions), batch(free)] so no device transposes are needed.
"""

import numpy as np

B = 512
IN = 2048
H = 4096
SH = 512          # hidden cols per core
NCORES = 8
KA = IN // 128    # 16  K-tiles for x contraction
KC = H // 128     # 32  K-tiles for h contraction
M = SH // 128     # 4   m-tiles per shard
F = H // 128      # 32  f-tiles for pred output
DT = 0.1
RB = 64           # router blocks
RSH = RB // NCORES  # 8 router blocks per core

_prog = None
MMDT = "bfloat16"   # matmul operand dtype: "bfloat16" or "float32r"


def _legalize_waits(nc, mybir, max_waits=1):
    """Split multi-wait instructions into single-wait NoOps.

    The walrus build here rejects >1 piggybacked sync wait on (at least)
    S3_LW-lowered matmuls and Drains.
    """
    ctr = 0
    n_split = 0
    for f in nc.m.functions:
        for blk in f.blocks:
            out = []
            for ins in blk.instructions:
                si = ins.sync_info
                if si is not None and len(si.on_wait) > max_waits:
                    waits = list(si.on_wait)
                    extra, keep = waits[:-max_waits], waits[-max_waits:]
                    for w in extra:
                        ctr += 1
                        nop = mybir.InstNoOp(name=f"waitnop-{ctr}")
                        nop.engine = ins.engine
                        nop.sync_info = mybir.SyncInfo(on_wait=[w], on_update=[])
                        out.append(nop)
                        n_split += 1
                    si.on_wait = keep
                out.append(ins)
            blk.instructions[:] = out
    return n_split


def _build_program():
    import concourse.bass as bass
    import concourse.mybir as mybir
    import concourse.tile as tile

    f32 = mybir.dt.float32
    f32r = mybir.dt.bfloat16 if MMDT == "bfloat16" else mybir.dt.float32r
    Alu = mybir.AluOpType
    Act = mybir.ActivationFunctionType

    nc = bass.Bass()

    awr_d = nc.dram_tensor("awr", [KA, 128, 1032], f32r, kind="ExternalInput")
    hr_d = nc.dram_tensor("hr", [KC, 128, 1024], f32r, kind="ExternalInput")
    p_d = nc.dram_tensor("p", [M, 128, H], f32r, kind="ExternalInput")
    hps_d = nc.dram_tensor("hps", [M, 128, B], f32, kind="ExternalInput")
    g_d = nc.dram_tensor("g", [M, 128, B], f32, kind="ExternalInput")
    eb_d = nc.dram_tensor("eb", [RSH, B], f32r, kind="ExternalInput")
    bias_d = nc.dram_tensor("bias", [128, 2 * M + 1], f32, kind="ExternalInput")
    hn_d = nc.dram_tensor("hn", [M, 128, B], f32, kind="ExternalOutput")
    pp_d = nc.dram_tensor("pp", [F, 128, B], f32, kind="ExternalOutput")

    with tile.TileContext(nc) as tc:
        with (
            tc.tile_pool(name="consts", bufs=1) as consts,
            tc.tile_pool(name="astream", bufs=3) as astream,
            tc.tile_pool(name="cstream", bufs=3) as cstream,
            tc.tile_pool(name="pres", bufs=1) as pres,
            tc.tile_pool(name="sens", bufs=1) as senspool,
            tc.tile_pool(name="hn", bufs=1) as hnpool,
            tc.tile_pool(name="tmp", bufs=3) as tmppool,
            tc.tile_pool(name="acc", bufs=4, space="PSUM") as acc_pool,
            tc.tile_pool(name="psb", bufs=2, space="PSUM") as psb_pool,
            tc.tile_pool(name="psr", bufs=1, space="PSUM") as psr_pool,
        ):
            # ---- constants ----
            eb_t = consts.tile([RSH, B], f32r, tag="eb")
            nc.sync.dma_start(eb_t[:], eb_d[:])
            bias_t = consts.tile([128, 2 * M + 1], f32, tag="bias")
            nc.sync.dma_start(bias_t[:], bias_d[:])
            hps_t = consts.tile([128, M, B], f32, tag="hps")
            g_t = consts.tile([128, M, B], f32, tag="g")
            for m in range(M):
                nc.sync.dma_start(hps_t[:, m, :], hps_d[m])
                nc.sync.dma_start(g_t[:, m, :], g_d[m])

            # ---- phase A: router + sensory accumulation over x K-tiles ----
            rg_ps = psr_pool.tile([RSH, B], f32, tag="rg")
            s_ps = [acc_pool.tile([128, B], f32, tag="acc", name=f"s_ps{i}") for i in range(M)]
            for k in range(KA):
                a_t = astream.tile([128, 1032], f32r, tag="awr")
                nc.sync.dma_start(a_t[:], awr_d[k])
                xt = a_t[:, 0:B]
                nc.tensor.matmul(rg_ps[:], a_t[:, 1024:1032], xt,
                                 start=(k == 0), stop=(k == KA - 1))
                for m in range(M):
                    nc.tensor.matmul(s_ps[m][:], a_t[:, B + 128 * m:B + 128 * (m + 1)],
                                     xt, start=(k == 0), stop=(k == KA - 1))

            # ---- phase B: sigmoid -> mask expand -> masked sensory drain ----
            rg32 = tmppool.tile([RSH, B], f32, tag="rg32")
            nc.scalar.activation(rg32[:], rg_ps[:], Act.Sigmoid,
                                 bias=bias_t[0:RSH, 2 * M:2 * M + 1], scale=1.0)
            rg_r = tmppool.tile([RSH, B], f32r, tag="rgr")
            nc.vector.tensor_copy(rg_r[:], rg32[:])

            sens = []
            for m in range(M):
                mask_ps = psb_pool.tile([128, B], f32, tag="mask")
                nc.tensor.matmul(mask_ps[:], eb_t[:, 128 * m:128 * (m + 1)], rg_r[:],
                                 start=True, stop=True)
                mask_sb = tmppool.tile([128, B], f32, tag="masksb")
                nc.scalar.activation(mask_sb[:], mask_ps[:], Act.Copy)
                s_sb = senspool.tile([128, B], f32, tag=f"sens{m}")
                # (x@W + bW) * mask
                nc.vector.scalar_tensor_tensor(
                    s_sb[:], s_ps[m][:], bias_t[:, m:m + 1], mask_sb[:],
                    op0=Alu.add, op1=Alu.mult)
                sens.append(s_sb)

            # ---- phase C: rec accumulation over h_prev K-tiles ----
            rec_ps = [acc_pool.tile([128, B], f32, tag="acc", name=f"rec_ps{i}") for i in range(M)]
            p_t = pres.tile([128, M, H], f32r, tag="p")
            for k in range(KC):
                c_t = cstream.tile([128, 1024], f32r, tag="hr")
                nc.sync.dma_start(c_t[:], hr_d[k])
                ht = c_t[:, 0:B]
                for m in range(M):
                    nc.tensor.matmul(rec_ps[m][:], c_t[:, B + 128 * m:B + 128 * (m + 1)],
                                     ht, start=(k == 0), stop=(k == KC - 1))
                # spread the 8MB P-slab load across phase C (1MB chunks)
                if k % 4 == 0:
                    chunk = k // 4  # 0..7
                    kb, h = divmod(chunk, 2)
                    nc.sync.dma_start(
                        p_t[:, kb, 2048 * h:2048 * (h + 1)],
                        p_d[kb][:, 2048 * h:2048 * (h + 1)])

            # ---- phase D: target, h_new ----
            hn_ts = []
            for m in range(M):
                tmp = tmppool.tile([128, B], f32, tag="dtmp")
                # (rec + bR) + sens
                nc.vector.scalar_tensor_tensor(
                    tmp[:], rec_ps[m][:], bias_t[:, M + m:M + m + 1], sens[m][:],
                    op0=Alu.add, op1=Alu.add)
                tgt = tmppool.tile([128, B], f32, tag="dtgt")
                nc.scalar.activation(tgt[:], tmp[:], Act.Tanh)
                d_sb = tmppool.tile([128, B], f32, tag="dd")
                nc.vector.tensor_sub(d_sb[:], tgt[:], hps_t[:, m, :])
                e_sb = tmppool.tile([128, B], f32, tag="de")
                nc.vector.tensor_mul(e_sb[:], d_sb[:], g_t[:, m, :])
                hn_sb = hnpool.tile([128, B], f32, tag=f"hn{m}")
                nc.vector.tensor_add(hn_sb[:], e_sb[:], hps_t[:, m, :])
                nc.sync.dma_start(hn_d[m], hn_sb[:])
                hn_r = hnpool.tile([128, B], f32r, tag=f"hnr{m}")
                nc.vector.tensor_copy(hn_r[:], hn_sb[:])
                hn_ts.append(hn_r)

            # ---- phase E: pred partial = P[shard,:]^T @ h_new^T_shard ----
            for f in range(F):
                pp_ps = acc_pool.tile([128, B], f32, tag="acc", name=f"pp_ps{f}")
                for kb in range(M):
                    nc.tensor.matmul(pp_ps[:], p_t[:, kb, 128 * f:128 * (f + 1)],
                                     hn_ts[kb][:], start=(kb == 0), stop=(kb == M - 1))
                pp_sb = tmppool.tile([128, B], f32, tag="ppsb", name=f"pp_sb{f}")
                nc.scalar.activation(pp_sb[:], pp_ps[:], Act.Copy)
                nc.sync.dma_start(pp_d[f], pp_sb[:])

    _legalize_waits(nc, mybir)
    nc.finalize()
    return nc


def _get_program():
    global _prog
    if _prog is None:
        _prog = _build_program()
    return _prog


def _densify(rows, cols, vals, n_rows, n_cols):
    flat = rows.astype(np.int64) * n_cols + cols.astype(np.int64)
    dense = np.bincount(flat, weights=vals.astype(np.float64),
                        minlength=n_rows * n_cols)
    return dense.astype(np.float32).reshape(n_rows, n_cols)


def kernel(x, h_prev, gate, W_vals, W_bias, R_vals, R_bias, P_vals, P_bias,
           router_w, router_b, tau, W_rows, W_cols, R_rows, R_cols,
           P_rows, P_cols):
    from concourse.bass_utils import run_bass_kernel_spmd

    if MMDT == "bfloat16":
        import ml_dtypes
        mmdt = ml_dtypes.bfloat16
    else:
        mmdt = np.float32

    x = np.asarray(x, np.float32)
    h_prev = np.asarray(h_prev, np.float32)
    gate = np.asarray(gate, np.float32)

    Wd = _densify(np.asarray(W_rows), np.asarray(W_cols), np.asarray(W_vals), IN, H)
    Rd = _densify(np.asarray(R_rows), np.asarray(R_cols), np.asarray(R_vals), H, H)
    Pd = _densify(np.asarray(P_rows), np.asarray(P_cols), np.asarray(P_vals), H, H)

    XT = np.ascontiguousarray(x.T).reshape(KA, 128, B)              # [16,128,512]
    HpT = np.ascontiguousarray(h_prev.T).reshape(KC, 128, B)        # [32,128,512]
    rwT = np.ascontiguousarray(np.asarray(router_w, np.float32).T)  # [2048, 64]
    dt_tau = (DT / np.asarray(tau, np.float32))                     # [4096]
    gate1 = gate.reshape(B)

    ebmat = np.zeros((RSH, B), np.float32)
    for j in range(RSH):
        ebmat[j, 64 * j:64 * (j + 1)] = 1.0

    in_maps = []
    for c in range(NCORES):
        sh = slice(SH * c, SH * (c + 1))
        w_slab = np.ascontiguousarray(Wd[:, sh]).reshape(KA, 128, SH)
        r_slab = np.ascontiguousarray(Rd[:, sh]).reshape(KC, 128, SH)
        p_slab = np.ascontiguousarray(Pd[sh, :]).reshape(M, 128, H)
        rw_slab = np.ascontiguousarray(
            rwT[:, RSH * c:RSH * (c + 1)]).reshape(KA, 128, RSH)
        awr = np.concatenate([XT, w_slab, rw_slab], axis=2).astype(mmdt)
        hr = np.concatenate([HpT, r_slab], axis=2).astype(mmdt)
        hps = np.ascontiguousarray(h_prev.T[sh]).reshape(M, 128, B)
        g = np.ascontiguousarray(
            np.outer(dt_tau[sh], gate1)).reshape(M, 128, B)
        bias = np.zeros((128, 2 * M + 1), np.float32)
        bias[:, 0:M] = np.asarray(W_bias, np.float32)[sh].reshape(M, 128).T
        bias[:, M:2 * M] = np.asarray(R_bias, np.float32)[sh].reshape(M, 128).T
        bias[0:RSH, 2 * M] = np.asarray(router_b, np.float32)[RSH * c:RSH * (c + 1)]
        in_maps.append({
            "awr": np.ascontiguousarray(awr),
            "hr": np.ascontiguousarray(hr),
            "p": p_slab.astype(mmdt),
            "hps": hps,
            "g": g,
            "eb": ebmat.astype(mmdt),
            "bias": bias,
        })

    nc = _get_program()
    res = run_bass_kernel_spmd(nc, in_maps, list(range(NCORES)))

    h_new = np.empty((B, H), np.float32)
    predT = np.zeros((H, B), np.float64)
    for c in range(NCORES):
        sh = slice(SH * c, SH * (c + 1))
        h_new[:, sh] = res.results[c]["hn"].reshape(SH, B).T
        predT += res.results[c]["pp"].reshape(H, B)
    pred = predT.T.astype(np.float32) + np.asarray(P_bias, np.float32)
    return (h_new, pred)


# revision 8
# speedup vs baseline: 1.5995x; 1.2656x over previous
"""Trainium2 Bass kernel for NeuromodulatedHolographicBrain.

Math (reference):
    r_gate  = sigmoid(x @ router_w.T + router_b)            # [B, 64]
    mask    = repeat(r_gate, 64, axis=1)                    # [B, H]
    sensory = (x @ W + bW) * mask                           # W from COO edges
    rec     = h_prev @ R + bR
    target  = tanh(sensory + rec)
    h_new   = h_prev + gate * (target - h_prev) * (DT/tau)
    pred    = h_new @ P + bP
    return (h_new, pred)

Strategy: densify the 1%-sparse edge-list weights on the host, then run
dense bf16 matmuls (fp32 PSUM accumulation) on the PE array. Hidden dim
(4096) is column-sharded across 8 cores (512 cols each): each core reads
x^T and h_prev^T in full, its own W/R column slabs and P row slab,
computes its h_new^T shard and a full [H, B] pred^T partial (contraction
over its h_new shard); the host sums the 8 partials. Everything on-chip
is in transposed layout [features(partitions), batch(free)], so no device
transposes are needed. All bulk DMAs are ~1 MiB with >=8KB per-partition
contiguous runs (partition-major DRAM layouts) for ~340 GB/s.
"""

import numpy as np

B = 512
IN = 2048
H = 4096
SH = 512          # hidden cols per core
NCORES = 8
KA = IN // 128    # 16  K-tiles for x contraction
KC = H // 128     # 32  K-tiles for h contraction
M = SH // 128     # 4   m-tiles per shard
F = H // 128      # 32  f-tiles for pred output
GA = KA // 4      # 4   awr groups (4 K-tiles per DMA)
GC = KC // 4      # 8   hr groups
GF = F // 4       # 8   pred output groups
DT = 0.1
RB = 64           # router blocks
RSH = RB // NCORES  # 8 router blocks per core
AW = B + SH + RSH   # 1032 awr row width
HW = B + SH         # 1024 hr row width

_prog = None
MMDT = "bfloat16"   # matmul operand dtype: "bfloat16" or "float32r"


def _legalize_waits(nc, mybir, max_waits=1):
    """Split multi-wait instructions into single-wait NoOps.

    The walrus build here rejects >1 piggybacked sync wait per instruction
    (seen on S3_LW-lowered matmuls and Drains). Run after TileContext
    exit, before nc.finalize().
    """
    ctr = 0
    n_split = 0
    for f in nc.m.functions:
        for blk in f.blocks:
            out = []
            for ins in blk.instructions:
                si = ins.sync_info
                if si is not None and len(si.on_wait) > max_waits:
                    waits = list(si.on_wait)
                    extra, keep = waits[:-max_waits], waits[-max_waits:]
                    for w in extra:
                        ctr += 1
                        nop = mybir.InstNoOp(name=f"waitnop-{ctr}")
                        nop.engine = ins.engine
                        nop.sync_info = mybir.SyncInfo(on_wait=[w], on_update=[])
                        out.append(nop)
                        n_split += 1
                    si.on_wait = keep
                out.append(ins)
            blk.instructions[:] = out
    return n_split


def _build_program():
    import concourse.bass as bass
    import concourse.mybir as mybir
    import concourse.tile as tile

    f32 = mybir.dt.float32
    mdt = mybir.dt.bfloat16 if MMDT == "bfloat16" else mybir.dt.float32r
    Alu = mybir.AluOpType
    Act = mybir.ActivationFunctionType

    nc = bass.Bass()

    # partition-major group layouts: one ~1MB DMA per group
    awr_d = nc.dram_tensor("awr", [GA, 128, 4 * AW], mdt, kind="ExternalInput")
    hr_d = nc.dram_tensor("hr", [GC, 128, 4 * HW], mdt, kind="ExternalInput")
    p_d = nc.dram_tensor("p", [M, 128, H], mdt, kind="ExternalInput")
    hps_d = nc.dram_tensor("hps", [128, M, B], f32, kind="ExternalInput")
    g_d = nc.dram_tensor("g", [128, M, B], f32, kind="ExternalInput")
    eb_d = nc.dram_tensor("eb", [RSH, B], mdt, kind="ExternalInput")
    bias_d = nc.dram_tensor("bias", [128, 2 * M + 1], f32, kind="ExternalInput")
    hn_d = nc.dram_tensor("hn", [128, M, B], f32, kind="ExternalOutput")
    pp_d = nc.dram_tensor("pp", [GF, 128, 4, B], f32, kind="ExternalOutput")

    with tile.TileContext(nc) as tc:
        with (
            tc.tile_pool(name="consts", bufs=1) as consts,
            tc.tile_pool(name="astream", bufs=2) as astream,
            tc.tile_pool(name="cstream", bufs=2) as cstream,
            tc.tile_pool(name="pres", bufs=1) as pres,
            tc.tile_pool(name="sens", bufs=1) as senspool,
            tc.tile_pool(name="recs", bufs=1) as recpool,
            tc.tile_pool(name="hn", bufs=1) as hnpool,
            tc.tile_pool(name="tmp", bufs=2) as tmppool,
            tc.tile_pool(name="outb", bufs=2) as outpool,
            tc.tile_pool(name="acc", bufs=4, space="PSUM") as acc_pool,
            tc.tile_pool(name="psb", bufs=2, space="PSUM") as psb_pool,
            tc.tile_pool(name="psr", bufs=1, space="PSUM") as psr_pool,
        ):
            # ---- small constants ----
            eb_t = consts.tile([RSH, B], mdt, tag="eb")
            nc.sync.dma_start(eb_t[:], eb_d[:])
            bias_t = consts.tile([128, 2 * M + 1], f32, tag="bias")
            nc.sync.dma_start(bias_t[:], bias_d[:])

            # ---- phase C: rec accumulation over h_prev K-tiles (first: no
            # dependencies, drains via plain copies so sensory can overlap) ----
            rec_ps = [acc_pool.tile([128, B], f32, tag="acc", name=f"rec_ps{i}")
                      for i in range(M)]
            p_t = pres.tile([128, M, H], mdt, tag="p")
            hps_t = consts.tile([128, M, B], f32, tag="hps")
            g_t = consts.tile([128, M, B], f32, tag="g")
            for c in range(GC):
                c_t = cstream.tile([128, 4, HW], mdt, tag="hr")
                nc.sync.dma_start(c_t[:], hr_d[c])
                for j in range(4):
                    k = 4 * c + j
                    ht = c_t[:, j, 0:B]
                    for m in range(M):
                        nc.tensor.matmul(
                            rec_ps[m][:], c_t[:, j, B + 128 * m:B + 128 * (m + 1)],
                            ht, start=(k == 0), stop=(k == KC - 1))
                # spread P-slab (4x1MB) + hps/g loads across phase C
                if c % 2 == 0:
                    nc.sync.dma_start(p_t[:, c // 2, :], p_d[c // 2])
                elif c == 1:
                    nc.sync.dma_start(hps_t[:], hps_d[:])
                elif c == 3:
                    nc.sync.dma_start(g_t[:], g_d[:])

            rec_sb = []
            for m in range(M):
                r_sb = recpool.tile([128, B], f32, tag=f"rec{m}", name=f"rec_sb{m}")
                nc.scalar.activation(r_sb[:], rec_ps[m][:], Act.Copy)
                rec_sb.append(r_sb)

            # ---- phase A: router + sensory accumulation over x K-tiles ----
            rg_ps = psr_pool.tile([RSH, B], f32, tag="rg")
            s_ps = [acc_pool.tile([128, B], f32, tag="acc", name=f"s_ps{i}")
                    for i in range(M)]
            for c in range(GA):
                a_t = astream.tile([128, 4, AW], mdt, tag="awr")
                nc.sync.dma_start(a_t[:], awr_d[c])
                for j in range(4):
                    k = 4 * c + j
                    xt = a_t[:, j, 0:B]
                    nc.tensor.matmul(rg_ps[:], a_t[:, j, B + SH:B + SH + RSH], xt,
                                     start=(k == 0), stop=(k == KA - 1))
                    for m in range(M):
                        nc.tensor.matmul(
                            s_ps[m][:], a_t[:, j, B + 128 * m:B + 128 * (m + 1)],
                            xt, start=(k == 0), stop=(k == KA - 1))

            # ---- phase B: sigmoid -> mask expand -> masked sensory drain ----
            rg32 = tmppool.tile([RSH, B], f32, tag="rg32")
            nc.scalar.activation(rg32[:], rg_ps[:], Act.Sigmoid,
                                 bias=bias_t[0:RSH, 2 * M:2 * M + 1], scale=1.0)
            rg_r = tmppool.tile([RSH, B], mdt, tag="rgr")
            nc.vector.tensor_copy(rg_r[:], rg32[:])

            sens = []
            for m in range(M):
                mask_ps = psb_pool.tile([128, B], f32, tag="mask",
                                        name=f"mask_ps{m}")
                nc.tensor.matmul(mask_ps[:], eb_t[:, 128 * m:128 * (m + 1)],
                                 rg_r[:], start=True, stop=True)
                mask_sb = tmppool.tile([128, B], f32, tag="masksb",
                                       name=f"mask_sb{m}")
                nc.scalar.activation(mask_sb[:], mask_ps[:], Act.Copy)
                s_sb = senspool.tile([128, B], f32, tag=f"sens{m}",
                                     name=f"sens_sb{m}")
                # (x@W + bW) * mask
                nc.vector.scalar_tensor_tensor(
                    s_sb[:], s_ps[m][:], bias_t[:, m:m + 1], mask_sb[:],
                    op0=Alu.add, op1=Alu.mult)
                sens.append(s_sb)

            # ---- phase D: target, h_new ----
            hn_sb = hnpool.tile([128, M, B], f32, tag="hnsb")
            hn_ts = []
            for m in range(M):
                tmp = tmppool.tile([128, B], f32, tag="dtmp", name=f"tmp{m}")
                # (rec + bR) + sens
                nc.vector.scalar_tensor_tensor(
                    tmp[:], rec_sb[m][:], bias_t[:, M + m:M + m + 1], sens[m][:],
                    op0=Alu.add, op1=Alu.add)
                tgt = tmppool.tile([128, B], f32, tag="dtgt", name=f"tgt{m}")
                nc.scalar.activation(tgt[:], tmp[:], Act.Tanh)
                d_sb = tmppool.tile([128, B], f32, tag="dd", name=f"d{m}")
                nc.vector.tensor_sub(d_sb[:], tgt[:], hps_t[:, m, :])
                e_sb = tmppool.tile([128, B], f32, tag="de", name=f"e{m}")
                nc.vector.tensor_mul(e_sb[:], d_sb[:], g_t[:, m, :])
                nc.vector.tensor_add(hn_sb[:, m, :], e_sb[:], hps_t[:, m, :])
                hn_r = hnpool.tile([128, B], mdt, tag=f"hnr{m}", name=f"hn_r{m}")
                nc.vector.tensor_copy(hn_r[:], hn_sb[:, m, :])
                hn_ts.append(hn_r)
            nc.sync.dma_start(hn_d[:], hn_sb[:])

            # ---- phase E: pred partial = P[shard,:]^T @ h_new^T_shard ----
            for fg in range(GF):
                pp_sb = outpool.tile([128, 4, B], f32, tag="ppsb",
                                     name=f"pp_sb{fg}")
                for j in range(4):
                    f = 4 * fg + j
                    pp_ps = acc_pool.tile([128, B], f32, tag="acc",
                                          name=f"pp_ps{f}")
                    for kb in range(M):
                        nc.tensor.matmul(pp_ps[:],
                                         p_t[:, kb, 128 * f:128 * (f + 1)],
                                         hn_ts[kb][:], start=(kb == 0),
                                         stop=(kb == M - 1))
                    nc.scalar.activation(pp_sb[:, j, :], pp_ps[:], Act.Copy)
                nc.sync.dma_start(pp_d[fg], pp_sb[:])

    _legalize_waits(nc, mybir)
    nc.finalize()
    return nc


def _get_program():
    global _prog
    if _prog is None:
        _prog = _build_program()
    return _prog


def _densify(rows, cols, vals, n_rows, n_cols):
    flat = rows.astype(np.int64) * n_cols + cols.astype(np.int64)
    dense = np.bincount(flat, weights=vals.astype(np.float64),
                        minlength=n_rows * n_cols)
    return dense.astype(np.float32).reshape(n_rows, n_cols)


def _group_pmajor(tiles, width):
    """[K,128,width] -> [K/4, 128, 4*width] with 4 K-tiles contiguous
    per partition row (>=8KB runs per partition per DMA)."""
    k = tiles.shape[0]
    return np.ascontiguousarray(
        tiles.reshape(k // 4, 4, 128, width).transpose(0, 2, 1, 3).reshape(
            k // 4, 128, 4 * width))


def kernel(x, h_prev, gate, W_vals, W_bias, R_vals, R_bias, P_vals, P_bias,
           router_w, router_b, tau, W_rows, W_cols, R_rows, R_cols,
           P_rows, P_cols):
    from concourse.bass_utils import run_bass_kernel_spmd

    if MMDT == "bfloat16":
        import ml_dtypes
        mmdt = ml_dtypes.bfloat16
    else:
        mmdt = np.float32

    x = np.asarray(x, np.float32)
    h_prev = np.asarray(h_prev, np.float32)
    gate = np.asarray(gate, np.float32)

    Wd = _densify(np.asarray(W_rows), np.asarray(W_cols), np.asarray(W_vals), IN, H)
    Rd = _densify(np.asarray(R_rows), np.asarray(R_cols), np.asarray(R_vals), H, H)
    Pd = _densify(np.asarray(P_rows), np.asarray(P_cols), np.asarray(P_vals), H, H)

    XT = np.ascontiguousarray(x.T).reshape(KA, 128, B)
    HpT = np.ascontiguousarray(h_prev.T).reshape(KC, 128, B)
    rwT = np.ascontiguousarray(np.asarray(router_w, np.float32).T)  # [2048, 64]
    dt_tau = (DT / np.asarray(tau, np.float32))                     # [4096]
    gate1 = gate.reshape(B)

    ebmat = np.zeros((RSH, B), np.float32)
    for j in range(RSH):
        ebmat[j, 64 * j:64 * (j + 1)] = 1.0
    ebmat = ebmat.astype(mmdt)

    in_maps = []
    for c in range(NCORES):
        sh = slice(SH * c, SH * (c + 1))
        w_slab = np.ascontiguousarray(Wd[:, sh]).reshape(KA, 128, SH)
        r_slab = np.ascontiguousarray(Rd[:, sh]).reshape(KC, 128, SH)
        p_slab = np.ascontiguousarray(Pd[sh, :]).reshape(M, 128, H)
        rw_slab = np.ascontiguousarray(
            rwT[:, RSH * c:RSH * (c + 1)]).reshape(KA, 128, RSH)
        awr = np.concatenate([XT, w_slab, rw_slab], axis=2).astype(mmdt)
        hr = np.concatenate([HpT, r_slab], axis=2).astype(mmdt)
        hps = np.ascontiguousarray(
            h_prev.T[sh].reshape(M, 128, B).transpose(1, 0, 2))
        g = np.ascontiguousarray(
            np.outer(dt_tau[sh], gate1).astype(np.float32)
            .reshape(M, 128, B).transpose(1, 0, 2))
        bias = np.zeros((128, 2 * M + 1), np.float32)
        bias[:, 0:M] = np.asarray(W_bias, np.float32)[sh].reshape(M, 128).T
        bias[:, M:2 * M] = np.asarray(R_bias, np.float32)[sh].reshape(M, 128).T
        bias[0:RSH, 2 * M] = np.asarray(router_b, np.float32)[RSH * c:RSH * (c + 1)]
        in_maps.append({
            "awr": _group_pmajor(awr, AW),
            "hr": _group_pmajor(hr, HW),
            "p": p_slab.astype(mmdt),
            "hps": hps,
            "g": g,
            "eb": ebmat,
            "bias": bias,
        })

    nc = _get_program()
    res = run_bass_kernel_spmd(nc, in_maps, list(range(NCORES)))

    h_new = np.empty((B, H), np.float32)
    predT = np.zeros((H, B), np.float64)
    for c in range(NCORES):
        sh = slice(SH * c, SH * (c + 1))
        # hn: [128, M, B] -> [M, 128, B] -> [SH, B] -> transpose
        h_new[:, sh] = res.results[c]["hn"].transpose(1, 0, 2).reshape(SH, B).T
        # pp: [GF, 128, 4, B] -> f-tile (fg, j) holds rows of pred^T
        predT += res.results[c]["pp"].transpose(0, 2, 1, 3).reshape(H, B)
    pred = predT.T.astype(np.float32) + np.asarray(P_bias, np.float32)
    return (h_new, pred)
